# revision 10
# baseline (speedup 1.0000x reference)
"""BERT (12-layer, C=768, B=4, T=1024, V=30522) forward pass on 8 Trainium2 cores.

Sharding: sequence-parallel over the 4096 tokens (512 tokens/core; core c owns
batch item c//2, sequence half c%2). Attention K/V for the other half of the
sequence is obtained with a pair-wise AllReduce (K_partner = K_sum - K_mine).
The LM head is vocab-sharded (3840 padded vocab rows/core) after an 8-way
AllGather of the final hidden states.

All matmuls run in fp32r (fp32 with 11-bit mantissa, full PE rate). Weights are
pre-transposed to [C_in, C_out] layout and pre-rounded to fp32r on the host.
Activations are kept feature-major [C, T] in SBUF; the residual stream stays
full fp32.
"""
import math
import sys

for _p in ("/opt/trn_rl_repo",):
    if _p not in sys.path:
        sys.path.insert(0, _p)

import numpy as np

import concourse.bass as bass
import concourse.mybir as mybir
import concourse.tile as tile
from concourse import bacc
from concourse.bass_utils import run_bass_kernel_spmd
from concourse.masks import make_identity

NC = 8          # cores
L = 12          # layers
C = 768         # model dim
H = 12          # heads
D = 64          # head dim
V = 30522       # vocab
B, T = 4, 1024
TQ = 512        # tokens per core
KC = C // 128   # 6 c-chunks
VP = 3840       # padded vocab rows per core (30 chunks of 128, 8 blocks of 480)
LN_EPS = 1e-5

F32 = mybir.dt.float32
F32R = mybir.dt.float32r
AF = mybir.ActivationFunctionType
ALU = mybir.AluOpType
PAIRS = [[0, 1], [2, 3], [4, 5], [6, 7]]
ALL8 = [[0, 1, 2, 3, 4, 5, 6, 7]]


def round_fp32r(x: np.ndarray) -> np.ndarray:
    """Round fp32 to fp32r (11-bit mantissa, RNE) like walrus fp32_to_fp32r."""
    bits = np.ascontiguousarray(x, dtype=np.float32).view(np.uint32)
    lsb = (bits >> 12) & 1
    rounded = (bits + 0x7FF + lsb) & np.uint32(0xFFFFF000)
    return rounded.view(np.float32)


def _layernorm(nc, tc, scr, tiny, psp, x_tiles, out_pool, out_tag, onesc, onesr,
               affine, lnw_ap, lnb_ap):
    """LN over the feature (partition) axis of feature-major x (6x[128,512]).

    x_tiles are fp32; output tiles are fp32r. Returns list of 6 output tiles.
    """
    xr = []
    sq = []
    for c in range(KC):
        xrt = scr.tile([128, TQ], F32R, tag="s512")
        nc.scalar.activation(xrt[:], x_tiles[c][:], AF.Copy)
        xr.append(xrt)
        sqt = scr.tile([128, TQ], F32R, tag="s512")
        nc.scalar.activation(sqt[:], x_tiles[c][:], AF.Square)
        sq.append(sqt)
    s1 = psp.tile([128, TQ], F32, tag="ps")
    for c in range(KC):
        nc.tensor.matmul(s1[:1, :], onesc[:], xr[c][:], start=(c == 0),
                         stop=(c == KC - 1))
    s2 = psp.tile([128, TQ], F32, tag="ps")
    for c in range(KC):
        nc.tensor.matmul(s2[:1, :], onesc[:], sq[c][:], start=(c == 0),
                         stop=(c == KC - 1))
    # f32 scratch slots: 0 = m, 1 = E[x^2]+eps, 2 = m^2 then sd, 3 = var+eps.
    # f32r tile (matmul-facing, every write is f32r): 0 = 1/sd, 1 = m/sd.
    lnt = tiny.tile([1, 4 * TQ], F32, tag="lnt", bufs=2)
    lntr = tiny.tile([1, 2 * TQ], F32R, tag="lntr", bufs=2)

    def sl(i):
        return lnt[0:1, i * TQ:(i + 1) * TQ]

    def sr(i):
        return lntr[0:1, i * TQ:(i + 1) * TQ]

    inv = 1.0 / C
    nc.vector.tensor_scalar_mul(sl(0), s1[:1, :], inv)              # m
    nc.vector.tensor_scalar(sl(1), s2[:1, :], inv, LN_EPS,
                            ALU.mult, ALU.add)                      # E[x^2] + eps
    nc.vector.tensor_mul(sl(2), sl(0), sl(0))                       # m^2
    nc.vector.tensor_sub(sl(3), sl(1), sl(2))                       # var + eps
    nc.scalar.activation(sl(2), sl(3), AF.Sqrt)                     # sd
    with nc.allow_low_precision(reason="fp32r rounding of 1/sd is intentional"):
        nc.vector.reciprocal(sr(0), sl(2))                          # 1/sd
    nc.vector.tensor_mul(sr(1), sl(0), sr(0).bitcast(F32))          # m/sd
    mb = psp.tile([128, TQ], F32, tag="ps")
    nc.tensor.matmul(mb[:], onesr[:], sr(1), start=True, stop=True)
    ib = psp.tile([128, TQ], F32, tag="ps")
    nc.tensor.matmul(ib[:], onesr[:], sr(0), start=True, stop=True)
    out = []
    for c in range(KC):
        t1 = scr.tile([128, TQ], F32, tag="s512")
        nc.vector.tensor_mul(t1[:], xr[c][:].bitcast(F32), ib[:])
        o = out_pool.tile([128, TQ], F32R, tag=out_tag)
        if affine:
            t2 = scr.tile([128, TQ], F32, tag="s512")
            nc.vector.tensor_sub(t2[:], t1[:], mb[:])
            nc.vector.tensor_scalar(o[:], t2[:], lnw_ap[:, c:c + 1],
                                    lnb_ap[:, c:c + 1], ALU.mult, ALU.add)
        else:
            nc.vector.tensor_sub(o[:], t1[:], mb[:])
        out.append(o)
    return out


def build(flags: tuple) -> bacc.Bacc:
    ln_affine, attn_bias, proj_bias, fc_bias, mproj_bias = flags
    nc = bacc.Bacc("TRN2", target_bir_lowering=False, num_devices=NC)

    # ---- external IO ----
    idx = nc.dram_tensor("idx", [TQ, 1], mybir.dt.int32, kind="ExternalInput")
    wte = nc.dram_tensor("wte", [V, C], F32, kind="ExternalInput")
    wpe = nc.dram_tensor("wpe", [TQ, C], F32, kind="ExternalInput")
    aw = nc.dram_tensor("aw", [L, C, 3 * C], F32R, kind="ExternalInput")
    pw = nc.dram_tensor("pw", [L, C, C], F32R, kind="ExternalInput")
    fw = nc.dram_tensor("fw", [L, C, 4 * C], F32R, kind="ExternalInput")
    mw = nc.dram_tensor("mw", [L, 4 * C, C], F32R, kind="ExternalInput")
    lmw = nc.dram_tensor("lmw", [C, VP], F32R, kind="ExternalInput")
    onesr_d = nc.dram_tensor("onesr", [1, 128], F32R, kind="ExternalInput")
    onesc_d = nc.dram_tensor("onesc", [128, 1], F32R, kind="ExternalInput")
    vones_d = nc.dram_tensor("vones", [128, H], F32R, kind="ExternalInput")
    if ln_affine:
        lnw_d = nc.dram_tensor("lnw", [2 * L + 1, 128, KC], F32, kind="ExternalInput")
        lnb_d = nc.dram_tensor("lnb", [2 * L + 1, 128, KC], F32, kind="ExternalInput")
    if attn_bias:
        abpp_d = nc.dram_tensor("abpp", [L, 128, 12], F32, kind="ExternalInput")
        abrow_d = nc.dram_tensor("abrow", [L, 1, 3 * C], F32R, kind="ExternalInput")
    if proj_bias:
        pb_d = nc.dram_tensor("pb", [L, 128, KC], F32, kind="ExternalInput")
    if fc_bias:
        fcb_d = nc.dram_tensor("fcb", [L, 128, 4 * KC], F32, kind="ExternalInput")
    if mproj_bias:
        mb_d = nc.dram_tensor("mb", [L, 128, KC], F32, kind="ExternalInput")
    logits = nc.dram_tensor("logits", [NC * TQ, VP], F32, kind="ExternalOutput")

    from contextlib import ExitStack

    with tile.TileContext(nc) as tc, ExitStack() as stack:
        cst = stack.enter_context(tc.tile_pool(name="cst", bufs=1))
        px = stack.enter_context(tc.tile_pool(name="px", bufs=6))
        tiny = stack.enter_context(tc.tile_pool(name="tiny", bufs=4))
        dram = stack.enter_context(tc.tile_pool(name="dram", bufs=2, space="DRAM"))

        onesr = cst.tile([1, 128], F32R, tag="onesr")
        nc.sync.dma_start(onesr[:], onesr_d[:])
        onesc = cst.tile([128, 1], F32R, tag="onesc")
        nc.sync.dma_start(onesc[:], onesc_d[:])
        vones = cst.tile([128, H], F32R, tag="vones")
        nc.sync.dma_start(vones[:], vones_d[:])
        ident = cst.tile([128, 128], F32, tag="ident")
        make_identity(nc, ident[:])
        if ln_affine:
            lnw_sb = cst.tile([128, (2 * L + 1) * KC], F32, tag="lnw")
            nc.sync.dma_start(
                lnw_sb[:], lnw_d[:].rearrange("a p c -> p (a c)"))
            lnb_sb = cst.tile([128, (2 * L + 1) * KC], F32, tag="lnb")
            nc.sync.dma_start(
                lnb_sb[:], lnb_d[:].rearrange("a p c -> p (a c)"))
        if attn_bias:
            abpp_sb = cst.tile([128, L * 12], F32, tag="abpp")
            nc.sync.dma_start(abpp_sb[:], abpp_d[:].rearrange("a p c -> p (a c)"))
            abrow_sb = cst.tile([1, L * 3 * C], F32R, tag="abrow")
            nc.sync.dma_start(abrow_sb[:], abrow_d[:].rearrange("a p c -> p (a c)"))
        if proj_bias:
            pb_sb = cst.tile([128, L * KC], F32, tag="pb")
            nc.sync.dma_start(pb_sb[:], pb_d[:].rearrange("a p c -> p (a c)"))
        if fc_bias:
            fcb_sb = cst.tile([128, L * 4 * KC], F32, tag="fcb")
            nc.sync.dma_start(fcb_sb[:], fcb_d[:].rearrange("a p c -> p (a c)"))
        if mproj_bias:
            mb_sb = cst.tile([128, L * KC], F32, tag="mb")
            nc.sync.dma_start(mb_sb[:], mb_d[:].rearrange("a p c -> p (a c)"))

        x_tiles = [px.tile([128, TQ], F32, tag="x", name=f"x{i}") for i in range(KC)]

        # ---- embedding: x = wte[idx] + wpe ----
        with tc.tile_pool(name="emb", bufs=5) as emb, \
             tc.tile_pool(name="embp", bufs=4, space="PSUM") as embp:
            for tt in range(4):
                it = emb.tile([128, 1], mybir.dt.int32, tag="it")
                nc.sync.dma_start(it[:], idx[128 * tt:128 * (tt + 1), :])
                g = emb.tile([128, C], F32, tag="g")
                nc.gpsimd.indirect_dma_start(
                    out=g[:], out_offset=None, in_=wte[:],
                    in_offset=bass.IndirectOffsetOnAxis(ap=it[:, :1], axis=0))
                wp = emb.tile([128, C], F32, tag="wp")
                nc.sync.dma_start(wp[:], wpe[128 * tt:128 * (tt + 1), :])
                xa = emb.tile([128, C], F32, tag="xa")
                nc.vector.tensor_add(xa[:], g[:], wp[:])
                for cc in range(KC):
                    pt = embp.tile([128, 128], F32, tag="pt")
                    nc.tensor.transpose(pt[:], xa[:, 128 * cc:128 * (cc + 1)], ident[:])
                    nc.vector.tensor_copy(
                        x_tiles[cc][:, 128 * tt:128 * (tt + 1)], pt[:])

        # ---- transformer layers ----
        for l in range(L):
            # ===== Scope A: LN1, QKV, KV exchange, attention, proj =====
            with tc.tile_pool(name="sa_scr", bufs=14) as scr, \
                 tc.tile_pool(name="sa_h", bufs=6) as ph, \
                 tc.tile_pool(name="sa_qk", bufs=18) as pqk, \
                 tc.tile_pool(name="sa_v", bufs=9) as pv, \
                 tc.tile_pool(name="sa_y", bufs=6) as py, \
                 tc.tile_pool(name="sa_w", bufs=4) as pw_pool, \
                 tc.tile_pool(name="sa_ps", bufs=8, space="PSUM") as psp:

                h = _layernorm(
                    nc, tc, scr, tiny, psp, x_tiles, ph, "h", onesc, onesr,
                    ln_affine,
                    lnw_sb[:, 2 * l * KC:(2 * l + 1) * KC] if ln_affine else None,
                    lnb_sb[:, 2 * l * KC:(2 * l + 1) * KC] if ln_affine else None)

                # --- QKV ---
                q_t, k_t = [], []
                for ob in range(2):  # 0 -> Q (o 0..767), 1 -> K (o 768..1535)
                    psl = [psp.tile([128, TQ], F32, tag="ps", name=f"psl{i}") for i in range(KC)]
                    for kc in range(KC):
                        wt = pw_pool.tile([128, C], F32R, tag="w768")
                        nc.sync.dma_start(
                            wt[:], aw[l, 128 * kc:128 * (kc + 1),
                                      C * ob:C * (ob + 1)])
                        for oc in range(KC):
                            nc.tensor.matmul(
                                psl[oc][:], wt[:, 128 * oc:128 * (oc + 1)],
                                h[kc][:], start=(kc == 0), stop=(kc == KC - 1))
                    for oc in range(KC):
                        dst = pqk.tile([128, TQ], F32R, tag="qk")
                        if attn_bias:
                            nc.vector.tensor_scalar_add(
                                dst[:], psl[oc][:],
                                abpp_sb[:, l * 12 + ob * KC + oc:
                                        l * 12 + ob * KC + oc + 1])
                        else:
                            nc.vector.tensor_copy(dst[:], psl[oc][:])
                        (q_t if ob == 0 else k_t).append(dst)

                # --- V (token-major, with ones column) ---
                v_t = []
                for tt in range(4):
                    vt = pv.tile([128, H * (D + 1)], F32R, tag="v")
                    va = psp.tile([128, TQ], F32, tag="ps")  # cols 0..512 heads 0-7
                    vb = psp.tile([128, TQ], F32, tag="ps")  # cols 0..256 heads 8-11
                    first = 0
                    if attn_bias:
                        brow = abrow_sb[:, l * 3 * C + 2 * C:l * 3 * C + 3 * C]
                        nc.tensor.matmul(va[:, :512],
                                         onesr[:, :128],
                                         brow[:, 0:512], start=True, stop=False)
                        nc.tensor.matmul(vb[:, :256],
                                         onesr[:, :128],
                                         brow[:, 512:768], start=True, stop=False)
                        first = 1
                    for kc in range(KC):
                        wt = pw_pool.tile([128, C], F32R, tag="w768")
                        nc.sync.dma_start(
                            wt[:], aw[l, 128 * kc:128 * (kc + 1), 2 * C:3 * C])
                        lhs = h[kc][:, 128 * tt:128 * (tt + 1)]
                        nc.tensor.matmul(va[:, :512], lhs, wt[:, 0:512],
                                         start=(kc == 0 and not first),
                                         stop=(kc == KC - 1))
                        nc.tensor.matmul(vb[:, :256], lhs, wt[:, 512:768],
                                         start=(kc == 0 and not first),
                                         stop=(kc == KC - 1))
                    vv = vt[:].rearrange("p (h e) -> p h e", e=D + 1)
                    nc.vector.tensor_copy(
                        vv[:, 0:8, 0:D],
                        va[:, :512].rearrange("p (h e) -> p h e", e=D))
                    nc.vector.tensor_copy(
                        vv[:, 8:12, 0:D],
                        vb[:, :256].rearrange("p (h e) -> p h e", e=D))
                    nc.vector.tensor_copy(vv[:, :, D:D + 1],
                                          vones[:].rearrange("p (h o) -> p h o", o=1))
                    v_t.append(vt)

                # --- pair exchange of K and V via AllReduce + subtract ---
                k_loc = dram.tile([C, TQ], F32, tag="kloc")
                k_sum = dram.tile([C, TQ], F32, tag="ksum")
                v_loc = dram.tile([TQ, H * (D + 1)], F32, tag="vloc")
                v_sum = dram.tile([TQ, H * (D + 1)], F32, tag="vsum")
                for kc in range(KC):
                    nc.sync.dma_start(k_loc[128 * kc:128 * (kc + 1), :],
                                      k_t[kc][:].bitcast(F32))
                for tt in range(4):
                    nc.sync.dma_start(v_loc[128 * tt:128 * (tt + 1), :],
                                      v_t[tt][:].bitcast(F32))
                nc.gpsimd.collective_compute(
                    "AllReduce", ALU.add, replica_groups=PAIRS,
                    ins=[k_loc.opt()], outs=[k_sum.opt()])
                nc.gpsimd.collective_compute(
                    "AllReduce", ALU.add, replica_groups=PAIRS,
                    ins=[v_loc.opt()], outs=[v_sum.opt()])
                k_r = []
                for kc in range(KC):
                    kst = scr.tile([128, TQ], F32, tag="s512", name=f"ks{kc}")
                    nc.sync.dma_start(kst[:], k_sum[128 * kc:128 * (kc + 1), :])
                    kr = pqk.tile([128, TQ], F32R, tag="qk", name=f"kr{kc}")
                    nc.vector.tensor_sub(kr[:], kst[:], k_t[kc][:].bitcast(F32))
                    k_r.append(kr)
                vs = scr.tile([128, 4, H * (D + 1)], F32, tag="vs", bufs=2)
                nc.sync.dma_start(vs[:], v_sum[:].rearrange("(a p) n -> p a n", p=128))
                v_r = []
                for tt in range(4):
                    vr = pv.tile([128, H * (D + 1)], F32R, tag="v")
                    nc.vector.tensor_sub(vr[:], vs[:, tt], v_t[tt][:].bitcast(F32))
                    v_r.append(vr)

                # --- attention (per head; local half then remote half) ---
                y_t = [py.tile([128, TQ], F32R, tag="y", name=f"y{i}") for i in range(KC)]
                for hh in range(H):
                    ct, ro = hh // 2, 64 * (hh % 2)
                    att = []
                    py_ps = psp.tile([128, TQ], F32, tag="ps")
                    for half in range(2):
                        ksrc = (k_t if half == 0 else k_r)[ct]
                        for sc in range(4):
                            ps_s = psp.tile([128, TQ], F32, tag="ps")
                            nc.tensor.matmul(
                                ps_s[:],
                                ksrc[ro:ro + D, 128 * sc:128 * (sc + 1)],
                                q_t[ct][ro:ro + D, :], start=True, stop=True)
                            at = scr.tile([128, TQ], F32R, tag="s512")
                            nc.scalar.activation(at[:], ps_s[:], AF.Exp,
                                                 scale=1.0 / math.sqrt(D))
                            att.append(at)
                    for j in range(8):
                        vsrc = (v_t if j < 4 else v_r)[j % 4]
                        nc.tensor.matmul(
                            py_ps[:D + 1, :],
                            vsrc[:, hh * (D + 1):(hh + 1) * (D + 1)],
                            att[j][:], start=(j == 0), stop=(j == 7))
                    rec = tiny.tile([1, TQ], F32R, tag="tinyr")
                    with nc.allow_low_precision(
                            reason="fp32r rounding of softmax recip"):
                        nc.vector.reciprocal(rec[:], py_ps[D:D + 1, :])
                    ps_r = psp.tile([128, TQ], F32, tag="ps")
                    nc.tensor.matmul(ps_r[:D, :], onesr[:, :D], rec[:],
                                     start=True, stop=True)
                    rsb = scr.tile([128, TQ], F32, tag="s512")
                    nc.scalar.activation(rsb[:D, :], ps_r[:D, :], AF.Copy)
                    nc.vector.tensor_mul(y_t[ct][ro:ro + D, :], py_ps[:D, :],
                                         rsb[:D, :])

                # --- proj + residual ---
                psl = [psp.tile([128, TQ], F32, tag="ps", name=f"psl{i}") for i in range(KC)]
                for kc in range(KC):
                    wt = pw_pool.tile([128, C], F32R, tag="w768")
                    nc.sync.dma_start(wt[:], pw[l, 128 * kc:128 * (kc + 1), :])
                    for oc in range(KC):
                        nc.tensor.matmul(
                            psl[oc][:], wt[:, 128 * oc:128 * (oc + 1)],
                            y_t[kc][:], start=(kc == 0), stop=(kc == KC - 1))
                for oc in range(KC):
                    if proj_bias:
                        nc.vector.scalar_tensor_tensor(
                            x_tiles[oc][:], psl[oc][:],
                            pb_sb[:, l * KC + oc:l * KC + oc + 1],
                            x_tiles[oc][:], ALU.add, ALU.add)
                    else:
                        nc.vector.tensor_add(x_tiles[oc][:], x_tiles[oc][:],
                                             psl[oc][:])

            # ===== Scope B: LN2, fc+gelu, mproj =====
            with tc.tile_pool(name="sb_scr", bufs=14) as scr, \
                 tc.tile_pool(name="sb_h", bufs=6) as ph, \
                 tc.tile_pool(name="sb_g", bufs=25) as pg, \
                 tc.tile_pool(name="sb_wf", bufs=4) as pwf, \
                 tc.tile_pool(name="sb_wm", bufs=4) as pwm, \
                 tc.tile_pool(name="sb_ps", bufs=8, space="PSUM") as psp:

                h2 = _layernorm(
                    nc, tc, scr, tiny, psp, x_tiles, ph, "h", onesc, onesr,
                    ln_affine,
                    lnw_sb[:, (2 * l + 1) * KC:(2 * l + 2) * KC] if ln_affine else None,
                    lnb_sb[:, (2 * l + 1) * KC:(2 * l + 2) * KC] if ln_affine else None)

                g_t = []
                for ob in range(3):  # 8 o-chunks each
                    psl = [psp.tile([128, TQ], F32, tag="ps", name=f"psl{i}") for i in range(8)]
                    for kc in range(KC):
                        wt = pwf.tile([128, 1024], F32R, tag="wf")
                        nc.sync.dma_start(
                            wt[:], fw[l, 128 * kc:128 * (kc + 1),
                                      1024 * ob:1024 * (ob + 1)])
                        for oc in range(8):
                            nc.tensor.matmul(
                                psl[oc][:], wt[:, 128 * oc:128 * (oc + 1)],
                                h2[kc][:], start=(kc == 0), stop=(kc == KC - 1))
                    for oc in range(8):
                        gt = pg.tile([128, TQ], F32R, tag="g")
                        ob_oc = ob * 8 + oc
                        bias = (fcb_sb[:, l * 4 * KC + ob_oc:l * 4 * KC + ob_oc + 1]
                                if fc_bias else 0.0)
                        nc.scalar.activation(gt[:], psl[oc][:], AF.Gelu_apprx_tanh,
                                             bias=bias)
                        g_t.append(gt)

                psl = [psp.tile([128, TQ], F32, tag="ps", name=f"psl{i}") for i in range(KC)]
                for k in range(4 * KC):
                    wt = pwm.tile([128, C], F32R, tag="wm")
                    nc.sync.dma_start(wt[:], mw[l, 128 * k:128 * (k + 1), :])
                    for oc in range(KC):
                        nc.tensor.matmul(
                            psl[oc][:], wt[:, 128 * oc:128 * (oc + 1)],
                            g_t[k][:], start=(k == 0), stop=(k == 4 * KC - 1))
                for oc in range(KC):
                    if mproj_bias:
                        nc.vector.scalar_tensor_tensor(
                            x_tiles[oc][:], psl[oc][:],
                            mb_sb[:, l * KC + oc:l * KC + oc + 1],
                            x_tiles[oc][:], ALU.add, ALU.add)
                    else:
                        nc.vector.tensor_add(x_tiles[oc][:], x_tiles[oc][:],
                                             psl[oc][:])

        # ---- final LN + AllGather of hidden states ----
        xf_loc = dram.tile([C, TQ], F32, tag="xfloc")
        xf_full = dram.tile([NC * C, TQ], F32, tag="xffull")
        with tc.tile_pool(name="fl_scr", bufs=14) as scr, \
             tc.tile_pool(name="fl_h", bufs=6) as ph, \
             tc.tile_pool(name="fl_ps", bufs=8, space="PSUM") as psp:
            xf = _layernorm(
                nc, tc, scr, tiny, psp, x_tiles, ph, "h", onesc, onesr,
                ln_affine,
                lnw_sb[:, 2 * L * KC:(2 * L + 1) * KC] if ln_affine else None,
                lnb_sb[:, 2 * L * KC:(2 * L + 1) * KC] if ln_affine else None)
            for kc in range(KC):
                nc.sync.dma_start(xf_loc[128 * kc:128 * (kc + 1), :],
                                  xf[kc][:].bitcast(F32))
        nc.gpsimd.collective_compute(
            "AllGather", ALU.bypass, replica_groups=ALL8,
            ins=[xf_loc.opt()], outs=[xf_full.opt()])

        # ---- LM head (vocab-sharded) ----
        with tc.tile_pool(name="lm_w", bufs=6) as plw, \
             tc.tile_pool(name="lm_xs", bufs=2) as pxs, \
             tc.tile_pool(name="lm_out", bufs=2) as plo, \
             tc.tile_pool(name="lm_ps", bufs=8, space="PSUM") as psp:
            lw = []
            for kc in range(KC):
                wt = plw.tile([128, VP], F32R, tag="lw")
                nc.sync.dma_start(wt[:], lmw[128 * kc:128 * (kc + 1), :])
                lw.append(wt)
            for g in range(NC):
                xs = pxs.tile([128, KC, TQ], F32R, tag="xs")
                nc.sync.dma_start(
                    xs[:],
                    xf_full[C * g:C * (g + 1), :]
                    .rearrange("(a p) n -> p a n", p=128).bitcast(F32R))
                for m in range(4):
                    osb = plo.tile([128, VP], F32, tag="lo")
                    for wave in range(2):
                        psl = [psp.tile([128, TQ], F32, tag="ps", name=f"psl{i}") for i in range(4)]
                        for kc in range(KC):
                            lhs = xs[:, kc, 128 * m:128 * (m + 1)]
                            for i in range(4):
                                nb = wave * 4 + i
                                nc.tensor.matmul(
                                    psl[i][:, :480],
                                    lhs, lw[kc][:, 480 * nb:480 * (nb + 1)],
                                    start=(kc == 0), stop=(kc == KC - 1))
                        for i in range(4):
                            nb = wave * 4 + i
                            nc.scalar.activation(
                                osb[:, 480 * nb:480 * (nb + 1)],
                                psl[i][:, :480], AF.Copy)
                    nc.sync.dma_start(
                        logits[TQ * g + 128 * m:TQ * g + 128 * (m + 1), :], osb[:])

    nc.compile()
    return nc


_CACHE = {}


def _get_nc(flags):
    if flags not in _CACHE:
        _CACHE[flags] = build(flags)
    return _CACHE[flags]


def kernel(idx, wte, wpe, ln1_w, ln1_b, attn_w, attn_b, proj_w, proj_b,
           ln2_w, ln2_b, fc_w, fc_b, mproj_w, mproj_b, lnf_w, lnf_b, lm_head_w):
    idx = np.asarray(idx)
    out_dtype = np.float32
    idx_flat = idx.reshape(B * T).astype(np.int32)
    wte = np.asarray(wte, dtype=np.float32)
    wpe = np.asarray(wpe, dtype=np.float32)[:T]

    ln_affine = not (
        np.all(ln1_w == 1) and np.all(ln1_b == 0) and np.all(ln2_w == 1)
        and np.all(ln2_b == 0) and np.all(lnf_w == 1) and np.all(lnf_b == 0))
    attn_bias = bool(np.any(attn_b != 0))
    proj_bias = bool(np.any(proj_b != 0))
    fc_bias = bool(np.any(fc_b != 0))
    mproj_bias = bool(np.any(mproj_b != 0))
    flags = (ln_affine, attn_bias, proj_bias, fc_bias, mproj_bias)
    nc = _get_nc(flags)

    # host-side layout prep: transpose weights to [C_in, C_out], round to fp32r
    aw_t = round_fp32r(np.ascontiguousarray(
        np.transpose(np.asarray(attn_w, np.float32), (0, 2, 1))))
    pw_t = round_fp32r(np.ascontiguousarray(
        np.transpose(np.asarray(proj_w, np.float32), (0, 2, 1))))
    fw_t = round_fp32r(np.ascontiguousarray(
        np.transpose(np.asarray(fc_w, np.float32), (0, 2, 1))))
    mw_t = round_fp32r(np.ascontiguousarray(
        np.transpose(np.asarray(mproj_w, np.float32), (0, 2, 1))))
    lm_pad = np.zeros((NC * VP, C), np.float32)
    lm_pad[:V] = np.asarray(lm_head_w, np.float32)
    lm_t = round_fp32r(np.ascontiguousarray(lm_pad.T))  # [C, NC*VP]

    common = {
        "wte": wte,
        "aw": aw_t, "pw": pw_t, "fw": fw_t, "mw": mw_t,
        "onesr": np.ones((1, 128), np.float32),
        "onesc": np.ones((128, 1), np.float32),
        "vones": np.ones((128, H), np.float32),
    }
    if ln_affine:
        def pp(w):  # [C] -> [128, KC]
            return np.ascontiguousarray(np.asarray(w, np.float32)
                                        .reshape(KC, 128).T)
        common["lnw"] = np.stack(
            [pp(w) for l in range(L) for w in (ln1_w[l], ln2_w[l])] + [pp(lnf_w)])
        common["lnb"] = np.stack(
            [pp(b) for l in range(L) for b in (ln1_b[l], ln2_b[l])] + [pp(lnf_b)])
    if attn_bias:
        common["abpp"] = np.ascontiguousarray(np.asarray(attn_b, np.float32)
                                              [:, :2 * C].reshape(L, 12, 128)
                                              .transpose(0, 2, 1))
        common["abrow"] = round_fp32r(np.asarray(attn_b, np.float32)
                                      .reshape(L, 1, 3 * C))
    if proj_bias:
        common["pb"] = np.ascontiguousarray(
            np.asarray(proj_b, np.float32).reshape(L, KC, 128).transpose(0, 2, 1))
    if fc_bias:
        common["fcb"] = np.ascontiguousarray(
            np.asarray(fc_b, np.float32).reshape(L, 4 * KC, 128).transpose(0, 2, 1))
    if mproj_bias:
        common["mb"] = np.ascontiguousarray(
            np.asarray(mproj_b, np.float32).reshape(L, KC, 128).transpose(0, 2, 1))

    in_maps = []
    for c in range(NC):
        m = dict(common)
        m["idx"] = idx_flat[TQ * c:TQ * (c + 1)].reshape(TQ, 1)
        m["wpe"] = np.ascontiguousarray(
            wpe[TQ * (c % 2):TQ * (c % 2) + TQ])
        m["lmw"] = np.ascontiguousarray(lm_t[:, VP * c:VP * (c + 1)])
        in_maps.append(m)

    res = run_bass_kernel_spmd(nc, in_maps, list(range(NC)))
    out = np.concatenate(
        [res.results[c]["logits"][:, :VP] for c in range(NC)], axis=1)
    return out[:, :V].reshape(B, T, V).astype(out_dtype)
